# revision 21
# baseline (speedup 1.0000x reference)
"""Trainium2 Bass kernel for CIN layer:
    out[b,c,d] = sum_{h,m} W[c, h*M+m] * xk[b,h,d] * x0[b,m,d] + bias[c]

Shapes (hardcoded): x0 [512,40,64] f32, xk [512,128,64] f32,
W [128,5120] f32, b [128] f32 -> out [512,128,64] f32.

Strategy: data-parallel over batch B across 8 cores (64 batches/core).
Per core, columns are the 64*64=4096 (b,d) pairs. The 5120-long (h,m)
contraction is split into 40 chunks of 128 rows with a mixed-radix
partition layout: chunk (g, j) covers m in the 8-wide group g (5 groups)
x h in the 16-wide block j (8 blocks); partition p holds
(m = 8g + p//16, h = 16j + p%16). Then
  outer[p, col] = xkrep_j[p, col] * x0bc_g[p, col]  (DVE TT, bf16 2x)
  psum[q] += w3[g,j][p,c].T @ outer[:, q*512:...]   (PE, accum 40 chunks)
where xkrep_j (xk h-block replicated 8x along partitions) and x0bc_g
(x0 m-group rows replicated 16x) are produced host-side (pure layout,
no arithmetic). W is host-gathered to match the chunk layout. Bias-add
is fused into the PSUM->SBUF eviction on ScalarE.

The kernel is DVE-bound: the outer-product tensor_muls are the
critical path (~245 G bf16 elem/s cap on tensor_tensor). Two builds:

reps=1 (single-shot, the correctness path): two column phases of
40x [128,2048] outers so compute starts after ~half the prologue
bytes; input DMAs are issued in first-use order for the j-outer chunk
sweep; w3 is split into 8 per-j-group tiles so a matmul's lhsT read
depends only on its own group's DMA; w3t0 loads first so PSUM-priming
matmuls warm the PE HAM clock-gate early.

reps>1 (hardware For_i loop, used for benching): a single pass of
40x [128,4096] outers (halves the DVE per-op overhead); all 8 PSUM
banks accumulate within one chunk sweep, and all evictions are rotated
to the START of the next iteration so the loop boundary never stalls
the DVE (epilogue eviction after the loop; banks primed before it).
Output is stored bf16 (host upcasts; ~4e-4 extra relative error).
"""

import contextlib

import numpy as np
import ml_dtypes

B, M, H, D, C = 512, 40, 128, 64, 128
N_CORES = 8
BC = B // N_CORES          # 64 batches per core
COLS = BC * D              # 4096 (b,d) columns per core
NG = 8                     # PSUM groups
GW = COLS // NG            # 512 columns per group
MG = 8                     # m-values per chunk group
NMG = M // MG              # 5 m-groups
HB = 128 // MG             # 16 h-values per block
NHB = H // HB              # 8 h-blocks
NCHUNK = NMG * NHB         # 40 contraction chunks

_cache = {}


def _build(reps=1, _nslot=7, _osb=4):
    import contextlib

    import concourse.bacc as bacc
    import concourse.mybir as mybir
    from concourse.tile import TileContext

    f32 = mybir.dt.float32
    bf16 = mybir.dt.bfloat16

    nc = bacc.Bacc("TRN2", debug=False, num_devices=N_CORES)

    xkr_d = nc.dram_tensor("xkrep_in", [NHB, 128, COLS], bf16, kind="ExternalInput")
    x0b_d = nc.dram_tensor("x0bc_in", [NMG, 128, COLS], bf16, kind="ExternalInput")
    w3_d = nc.dram_tensor("w3_in", [NCHUNK, 128, C], bf16, kind="ExternalInput")
    bias_d = nc.dram_tensor("bias_in", [C, 1], f32, kind="ExternalInput")
    out_d = nc.dram_tensor("out", [BC, C, D], bf16, kind="ExternalOutput")

    rotate = reps > 1

    with TileContext(nc) as tc:
        with (
            tc.tile_pool(name="const", bufs=1) as cpool,
            tc.tile_pool(name="work", bufs=6) as wpool,
            tc.tile_pool(name="outp", bufs=_osb) as opool,
            tc.tile_pool(name="psum", bufs=1, space="PSUM") as ppool,
        ):
            # ---- load constants / replicated operand tiles ----
            # Each tile is loaded as two half-column DMAs, all phase-0
            # halves first (in rough first-use order), so phase 0 of the
            # main loop can start after ~half the prologue bytes; Tile's
            # subtile dependency tracking lets the half-column TT reads
            # wait only on their half's DMA.
            HC = COLS // 2
            # w3 lives in 8 per-j-group tiles (5 chunks each) so a
            # matmul's lhsT read depends only on its own group's DMA
            # (whole-tile dependency on a single big w3 tile was gating
            # the first matmuls on the last w3 bytes).
            w3ts = [
                cpool.tile([128, NMG * C], bf16, name=f"w3t{j}", tag=f"w3t{j}")
                for j in range(NHB)
            ]
            w3_ap = w3_d.ap().rearrange("k p c -> p k c")

            bias_sb = cpool.tile([128, 1], f32)
            nc.sync.dma_start(out=bias_sb, in_=bias_d.ap())

            xkreps = [None] * NHB
            x0bcs = [None] * NMG
            load_order = [("x", 0), ("0", 0), ("x", 1), ("x", 2), ("0", 1),
                          ("x", 3), ("x", 4), ("0", 2), ("x", 5), ("x", 6),
                          ("0", 3), ("x", 7), ("0", 4)]
            for kind, i in load_order:
                if kind == "x":
                    xkreps[i] = cpool.tile(
                        [128, COLS], bf16, name=f"xkr{i}", tag=f"xkr{i}"
                    )
                else:
                    x0bcs[i] = cpool.tile(
                        [128, COLS], bf16, name=f"x0b{i}", tag=f"x0b{i}"
                    )
            # DMA issue order = first-use order for the j-outer chunk
            # sweep: chunk group j needs xkr[j], w3t[j] and the five x0b
            # tiles; {xkr0, x0b0..4} unblocks the first group and each
            # later group waits only on one more xkr half + w3 piece.
            # Phase-1 halves are emitted lazily (after phase-0 compute)
            # on the single-shot path so compute never waits on them.
            dma_order = [("x", 0)] + [("0", i) for i in range(NMG)]
            dma_order += [("x", i) for i in range(1, NHB)]

            def issue_w3_piece(j):
                nc.sync.dma_start(
                    out=w3ts[j],
                    in_=w3_ap[:, j * NMG:(j + 1) * NMG, :],
                )

            def issue_input_dmas(ph, with_w3=False, skip=0):
                nxt = 1
                for pos, (kind, i) in enumerate(dma_order):
                    if pos < skip:
                        continue
                    tile_, src = (
                        (xkreps[i], xkr_d.ap()[i])
                        if kind == "x"
                        else (x0bcs[i], x0b_d.ap()[i])
                    )
                    nc.sync.dma_start(
                        out=tile_[:, ph * HC:(ph + 1) * HC],
                        in_=src[:, ph * HC:(ph + 1) * HC],
                    )
                    if with_w3 and pos >= 5 and nxt < NHB:
                        issue_w3_piece(nxt)
                        nxt += 1
                if with_w3:
                    while nxt < NHB:
                        issue_w3_piece(nxt)
                        nxt += 1

            # w3t0 goes first so the PSUM-priming matmuls (which warm
            # the PE's HAM clock-gate) can start ~3us in, not after all
            # the input halves.
            issue_w3_piece(0)
            issue_input_dmas(0, with_w3=True)
            if rotate:
                issue_input_dmas(1)

            GK = 5
            NSLOT = _nslot
            HCOL = COLS // 2
            out_ap = out_d.ap().rearrange("b c d -> c b d")
            bpg = BC // NG

            psums = []
            for q in range(NG):
                ps = ppool.tile([128, GW], f32, name=f"ps{q}", tag=f"ps{q}")
                psums.append(ps)

            # Prime all psum banks (so the rotated evictions never read
            # an unwritten bank) and warm the PE's HAM clock-gate while
            # the prologue DMAs are in flight. w3t0 slices serve as
            # dummy operands to avoid adding an SBUF allocation (the DVE
            # outer ops are sensitive to operand tile addresses).
            for _ in range(2):
                for q in range(NG):
                    nc.tensor.matmul(
                        psums[q], lhsT=w3ts[0][:, :128], rhs=w3ts[0][:, :GW],
                        start=True, stop=True,
                    )

            def compute_phase(tag_u, ph):
                # 2-phase path (single-shot): half-column outers so work
                # can start after ~half the prologue bytes. MMs issued in
                # groups of GK chunks, bank-major inside the group, so
                # the PE stays on one PSUM bank for GK consecutive
                # matmuls (bank cycling measurably degrades PE
                # throughput).
                for k0 in range(0, NCHUNK, GK):
                    outers = []
                    for k in range(k0, k0 + GK):
                        j, g = divmod(k, NMG)
                        outer = wpool.tile(
                            [128, HCOL], bf16, name=f"outer{tag_u}_{ph}_{k}",
                            tag=f"outer{k % NSLOT}", bufs=1,
                        )
                        nc.vector.tensor_mul(
                            outer,
                            xkreps[j][:, ph * HCOL:(ph + 1) * HCOL],
                            x0bcs[g][:, ph * HCOL:(ph + 1) * HCOL],
                        )
                        outers.append(outer)
                    for ql in range(NG // 2):
                        qb = ph * (NG // 2) + ql
                        for i, k in enumerate(range(k0, k0 + GK)):
                            nc.tensor.matmul(
                                psums[qb],
                                lhsT=w3ts[k // NMG][:, (k % NMG) * C:
                                                    (k % NMG + 1) * C],
                                rhs=outers[i][:, ql * GW:(ql + 1) * GW],
                                start=(k == 0),
                                stop=(k == NCHUNK - 1),
                            )

            def compute_wide(tag_u):
                # 1-pass path (steady-state loop): 4096-wide outers halve
                # the DVE per-op overhead (40 ops instead of 80); all 8
                # PSUM banks accumulate within the single chunk sweep.
                for k0 in range(0, NCHUNK, GK):
                    outers = []
                    for k in range(k0, k0 + GK):
                        j, g = divmod(k, NMG)
                        outer = wpool.tile(
                            [128, COLS], bf16, name=f"outer{tag_u}_{k}",
                            tag=f"outer{k % NSLOT}", bufs=1,
                        )
                        nc.vector.tensor_mul(outer, xkreps[j], x0bcs[g])
                        outers.append(outer)
                    for qb in range(NG):
                        for i, k in enumerate(range(k0, k0 + GK)):
                            nc.tensor.matmul(
                                psums[qb],
                                lhsT=w3ts[k // NMG][:, (k % NMG) * C:
                                                    (k % NMG + 1) * C],
                                rhs=outers[i][:, qb * GW:(qb + 1) * GW],
                                start=(k == 0),
                                stop=(k == NCHUNK - 1),
                            )

            def evict_banks(tag_u, banks):
                for qb in banks:
                    out_sb = opool.tile(
                        [128, GW], bf16, name=f"osb{tag_u}_{qb}", tag="osb"
                    )
                    nc.scalar.activation(
                        out_sb,
                        psums[qb],
                        mybir.ActivationFunctionType.Identity,
                        bias=bias_sb[:, 0:1],
                        scale=1.0,
                    )
                    nc.sync.dma_start(
                        out=out_ap[:, qb * bpg:(qb + 1) * bpg, :], in_=out_sb
                    )

            loop_ctx = (
                tc.For_i(
                    0, reps, 1,
                    hint_engines=(mybir.EngineType.PE,),
                    staggered_reset=True,
                )
                if reps > 1
                else contextlib.nullcontext()
            )
            with loop_ctx:
                if rotate:
                    # evictions of the previous iteration's banks overlap
                    # this iteration's compute instead of serializing at
                    # the loop boundary
                    evict_banks("r", range(NG))
                    compute_wide("u")
                else:
                    compute_phase("u", 0)
                    issue_input_dmas(1)
                    evict_banks("u", range(NG // 2))
                    compute_phase("u", 1)
                    evict_banks("u", range(NG // 2, NG))
            if rotate:
                evict_banks("e", range(NG))

    nc.compile()
    return nc


def _prep_host(x0, xk, W, b):
    """Host-side layout prep (no arithmetic): shard, transpose, replicate."""
    part = np.arange(128)
    hh = (part % HB)[None, :] + HB * np.arange(NHB)[:, None]   # [NHB, 128]
    mm = (part // HB)[None, :] + MG * np.arange(NMG)[:, None]  # [NMG, 128]

    Wr = W.reshape(C, H, M)
    w3 = np.empty((NCHUNK, 128, C), ml_dtypes.bfloat16)
    for j in range(NHB):
        for g in range(NMG):
            w3[j * NMG + g] = Wr[:, hh[j], mm[g]].T.astype(ml_dtypes.bfloat16)
    bias = np.ascontiguousarray(b.reshape(C, 1)).astype(np.float32)

    in_maps = []
    for k in range(N_CORES):
        x0s = x0[k * BC:(k + 1) * BC]            # [BC, M, D]
        xks = xk[k * BC:(k + 1) * BC]            # [BC, H, D]
        xk2 = (
            np.ascontiguousarray(xks.transpose(1, 0, 2))
            .reshape(H, COLS)
            .astype(ml_dtypes.bfloat16)
        )
        x02 = (
            np.ascontiguousarray(x0s.transpose(1, 0, 2))
            .reshape(M, COLS)
            .astype(ml_dtypes.bfloat16)
        )
        in_maps.append(
            {
                "xkrep_in": np.ascontiguousarray(xk2[hh]),
                "x0bc_in": np.ascontiguousarray(x02[mm]),
                "w3_in": w3,
                "bias_in": bias,
            }
        )
    return in_maps


def _run(in_maps, **kwargs):
    from concourse import bass_utils

    if "nc" not in _cache:
        _cache["nc"] = _build()
    return bass_utils.run_bass_kernel_spmd(
        _cache["nc"], in_maps, core_ids=list(range(N_CORES)), **kwargs
    )


def kernel(x0, xk, W, b, _bench=[None]):
    x0 = np.asarray(x0, dtype=np.float32)
    xk = np.asarray(xk, dtype=np.float32)
    W = np.asarray(W, dtype=np.float32)
    b = np.asarray(b, dtype=np.float32)
    in_maps = _prep_host(x0, xk, W, b)
    res = _run(in_maps)
    _bench[0] = res
    out = np.concatenate([r["out"] for r in res.results], axis=0)
    return out.astype(np.float32, copy=False)
